# revision 34
# baseline (speedup 1.0000x reference)
"""Trainium2 Bass kernel for nn_CAB.

Reference computation:
    interaction = A^T A            (C, C)
    CAG   = softmax(interaction, axis=-1)
    E     = A + beta * (A @ CAG^T) (M, C)
    returns (E, CAG)

Strategy (8 NeuronCores):
  - Shard A row-wise: 32768 rows per core.
  - DMA layout: rows swizzled "(n p s) d" so each partition reads 16 KiB
    contiguous per 2 MiB chunk (near-line-rate descriptors). Row grouping
    into 128-row subtiles is arbitrary for both A^T A and the row-wise
    pass 2, so the swizzle needs no fixup anywhere.
  - Pass 1: load chunk (exact f32) -> cast each 128-row subtile to f32r
    (DVE/ACT alternating; the BIR verifier requires a rounding producer
    for f32r matmul inputs) -> accumulate partial interaction on the PE
    with f32r single-pass matmuls (4x the fp32 rate).
  - Tiny AllReduce barrier up front absorbs cross-core launch skew so
    the real AllReduces wait less; interaction is all-reduced in two
    pipelined halves (after chunk 5 and after chunk 15) so most of the
    collective latency hides under pass-1 compute.
  - Softmax on-chip, replicated: -max -> Exp(bias=-max, fused row-sum)
    -> reciprocal -> scale. Bc = beta * CAG^T via PE block transposes.
  - Pass 2 (chunks in reverse): last RESIDENT chunks reuse the pass-1
    SBUF tiles (no re-read); rest re-stream. Per subtile: cast A_sub to
    bf16 (GpSimd, otherwise idle), PE-transpose it (bf16 transposes run
    2x f32), delta = atT-matmul with bf16 Bc into f32 PSUM, then
    E_sub = A_sub + delta on DVE in exact f32 (bit-exact when beta == 0,
    since Bc is then exactly zero), store E.
"""

import numpy as np

M, C = 262144, 256
N_CORES = 8
M_SHARD = M // N_CORES  # 32768
P = 128
CHUNK_SUB = 16  # 128-row subtiles per chunk -> 2 MiB chunks
N_CHUNKS = M_SHARD // (P * CHUNK_SUB)  # 16
AR_BOUNDS = (4, 11, 16)  # chunk boundaries of the chained all-reduces
RESIDENT = 8  # trailing pass-1 chunks kept in SBUF for pass 2

_CACHED_NC = None


def _body(nc, tc, a_in, beta_in, e_out, cag_out):
    from contextlib import ExitStack

    import concourse.mybir as mybir
    from concourse.masks import make_identity

    f32 = mybir.dt.float32
    f32r = mybir.dt.float32r
    bf16 = mybir.dt.bfloat16

    a_view = a_in.rearrange("(n p s) d -> n p s d", p=P, s=CHUNK_SUB)
    e_view = e_out.rearrange("(n p s) d -> n p s d", p=P, s=CHUNK_SUB)
    cag_view = cag_out.rearrange("(b p) d -> b p d", p=P)
    rg = [list(range(N_CORES))]

    with ExitStack() as ctx:
        a1p = ctx.enter_context(tc.tile_pool(name="a1p", bufs=RESIDENT))
        asrp = ctx.enter_context(tc.tile_pool(name="asrp", bufs=8))
        a2p = ctx.enter_context(tc.tile_pool(name="a2p", bufs=1))
        ep = ctx.enter_context(tc.tile_pool(name="ep", bufs=1))
        smp = ctx.enter_context(tc.tile_pool(name="smp", bufs=1))
        attp = ctx.enter_context(tc.tile_pool(name="attp", bufs=8))
        ps_int = ctx.enter_context(tc.tile_pool(name="ps_int", bufs=1, space="PSUM"))
        ps_tp = ctx.enter_context(tc.tile_pool(name="ps_tp", bufs=3, space="PSUM"))
        ps_d = ctx.enter_context(tc.tile_pool(name="ps_d", bufs=3, space="PSUM"))
        dramp = ctx.enter_context(tc.tile_pool(name="dramp", bufs=1, space="DRAM"))

        ident = smp.tile([P, P], f32, name="ident")
        make_identity(nc, ident)
        beta_sb = smp.tile([P, 1], f32, name="beta_sb")
        nc.sync.dma_start(beta_sb, beta_in)
        # beta's raw bits on every engine that branches: 0x0 iff beta == +/-0.0
        # (-0.0 takes the general path, which is still correct).
        E = mybir.EngineType
        beta_bits = nc.values_load(
            beta_sb.bitcast(mybir.dt.int32)[0:1, 0:1],
            engines=[E.SP, E.PE, E.DVE, E.Activation],
        )

        # ---- pass 1: partial interaction = A_shard^T @ A_shard ----
        # The slices share PSUM slots (tag per block); each slice's first
        # matmul just waits for the previous slice's epilogue copy. Three
        # chained all-reduces keep ncfw busy continuously, so the last one
        # starts without its ~20 us idle-wake penalty.
        n_ar = len(AR_BOUNDS)
        int_ps = {
            (h, b): ps_int.tile([P, C], f32, name=f"int_{h}{b}", tag=f"int_{b}")
            for h in range(n_ar)
            for b in range(2)
        }
        bounce = {
            (h, io): dramp.tile([C, C], f32, name=f"bounce_{h}{io}")
            for h in range(n_ar)
            for io in ("i", "o")
        }
        a1_tiles = []
        load_insts = []
        tail_stores = []
        ar_starts = (0,) + AR_BOUNDS[:-1]
        ar_lasts = tuple(b - 1 for b in AR_BOUNDS)
        for n in range(N_CHUNKS):
            h = min(i for i, b in enumerate(AR_BOUNDS) if n < b)
            at = a1p.tile([P, CHUNK_SUB, C], f32, name=f"a1_{n}", tag="a1")
            load_insts.append(nc.sync.dma_start(at, a_view[n]))
            a1_tiles.append(at)
            # beta == 0 fast path: E is exactly A; write it now, straight from
            # the pass-1 tile (skipped at runtime when beta != 0). The scalar
            # HWDGE ring carries only these stores during pass 1 (its compute
            # is all on DVE), so holding the ring never stalls real work.
            st = nc.scalar.dma_start(e_view[n], at, cond=beta_bits == 0)
            tail_stores.append((n, st))
            for s in range(CHUNK_SUB):
                asr = asrp.tile([P, C], f32r, name=f"asr_{n}_{s}", tag="asr")
                nc.vector.tensor_copy(asr, at[:, s])
                first = n in ar_starts and s == 0
                last = n in ar_lasts and s == CHUNK_SUB - 1
                for blk in range(2):
                    nc.tensor.matmul(
                        int_ps[h, blk][:],
                        lhsT=asr[:, blk * P : (blk + 1) * P],
                        rhs=asr,
                        start=first,
                        stop=last,
                    )
            if n in ar_lasts:
                # epilogue for this slice: PSUM -> SBUF -> DRAM bounce -> AR
                int_sb = [
                    smp.tile([P, C], f32, name=f"int_sb{h}{b}", tag=f"int_sb{h}{b}")
                    for b in range(2)
                ]
                nc.vector.tensor_copy(int_sb[0], int_ps[h, 0])
                nc.vector.tensor_copy(int_sb[1], int_ps[h, 1])
                bv = bounce[h, "i"].rearrange("(b p) d -> b p d", p=P)
                nc.sync.dma_start(bv[0], int_sb[0])
                nc.sync.dma_start(bv[1], int_sb[1])
                nc.gpsimd.collective_compute(
                    "AllReduce", mybir.AluOpType.add, replica_groups=rg,
                    ins=[bounce[h, "i"].opt()], outs=[bounce[h, "o"].opt()],
                )

        # The last RESIDENT stores wait for the last load: loads (which gate
        # the final all-reduce) get the DMA bandwidth first, and the held
        # stores backfill the all-reduce latency at the end.
        from concourse.bass import _add_dep_helper

        for n, st in tail_stores:
            if n >= N_CHUNKS - RESIDENT:
                _add_dep_helper(
                    st.ins,
                    load_insts[-1].ins,
                    sync=True,
                    reason="hold tail E:=A stores behind the last pass-1 load",
                )

        # ---- combine slices + softmax -> CAG (replicated on every core) ----
        cag_sb = []
        for b in range(2):
            iah = []
            for h in range(n_ar):
                t = smp.tile([P, C], f32, name=f"iah{h}{b}", tag=f"iah{h}{b}")
                nc.sync.dma_start(t, bounce[h, "o"].rearrange("(b p) d -> b p d", p=P)[b])
                iah.append(t)
            ia = smp.tile([P, C], f32, name=f"ia{b}", tag=f"ia{b}")
            nc.vector.tensor_add(ia, iah[0], iah[1])
            for h in range(2, n_ar):
                nc.vector.tensor_add(ia, ia, iah[h])
            negmax = smp.tile([P, 1], f32, name=f"negmax{b}", tag=f"negmax{b}")
            nc.vector.reduce_max(negmax, ia, axis=mybir.AxisListType.X, negate=True)
            prob = smp.tile([P, C], f32, name=f"prob{b}", tag=f"prob{b}")
            rs = smp.tile([P, 1], f32, name=f"rs{b}", tag=f"rs{b}")
            nc.scalar.activation(
                prob,
                ia,
                mybir.ActivationFunctionType.Exp,
                bias=negmax,
                scale=1.0,
                accum_out=rs,
            )
            rec = smp.tile([P, 1], f32, name=f"rec{b}", tag=f"rec{b}")
            nc.vector.reciprocal(rec, rs)
            cag = smp.tile([P, C], f32, name=f"cag{b}", tag=f"cag{b}")
            nc.vector.tensor_scalar_mul(cag, prob, rec)
            nc.sync.dma_start(cag_view[b], cag)
            cag_sb.append(cag)

        # ---- pass 2 (general path, branched over when beta == 0) ----
        # E = A + A @ Bc with Bc = beta * CAG^T; chunks reversed so the tail
        # reuses the resident pass-1 SBUF tiles.
        with tc.If(beta_bits != 0):
            bc = []
            for kb in range(2):
                tp = ps_tp.tile([P, C], f32, name=f"cagT{kb}", tag="tp")
                for cb in range(2):
                    nc.tensor.transpose(
                        tp[:, cb * P : (cb + 1) * P],
                        cag_sb[cb][:, kb * P : (kb + 1) * P],
                        ident,
                    )
                bctf = smp.tile([P, C], f32, name=f"bcf{kb}", tag=f"bcf{kb}")
                nc.scalar.mul(bctf, tp, mul=beta_sb)
                bct = smp.tile([P, C], bf16, name=f"bc{kb}", tag=f"bc{kb}")
                nc.vector.tensor_copy(bct, bctf)
                bc.append(bct)

            for n in reversed(range(N_CHUNKS)):
                if n >= N_CHUNKS - RESIDENT:
                    at = a1_tiles[n]
                else:
                    at = a2p.tile([P, CHUNK_SUB, C], f32, name=f"a2_{n}", tag="a2")
                    nc.sync.dma_start(at, a_view[n])
                et = ep.tile([P, CHUNK_SUB, C], f32, name=f"e_{n}", tag="e")
                for s in range(CHUNK_SUB):
                    asub = at[:, s]
                    tp = ps_tp.tile([P, C], f32, name=f"atT_{n}_{s}", tag="tp")
                    for kb in range(2):
                        nc.tensor.transpose(
                            tp[:, kb * P : (kb + 1) * P],
                            asub[:, kb * P : (kb + 1) * P],
                            ident,
                        )
                    att = attp.tile([P, C], bf16, name=f"att_{n}_{s}", tag="att")
                    nc.scalar.copy(att, tp)
                    dps = ps_d.tile([P, C], f32, name=f"d_{n}_{s}", tag="d")
                    nc.tensor.matmul(
                        dps, lhsT=att[:, 0:P], rhs=bc[0], start=True, stop=False
                    )
                    nc.tensor.matmul(
                        dps, lhsT=att[:, P:C], rhs=bc[1], start=False, stop=True
                    )
                    nc.vector.tensor_add(et[:, s], asub, dps)
                nc.sync.dma_start(e_view[n], et)


def build_nc():
    global _CACHED_NC
    if _CACHED_NC is not None:
        return _CACHED_NC

    import concourse.mybir as mybir
    import concourse.tile as tile
    from concourse import bacc

    f32 = mybir.dt.float32
    nc = bacc.Bacc(
        "TRN2", target_bir_lowering=False, debug=False, num_devices=N_CORES
    )
    a_in = nc.dram_tensor("a_in", [M_SHARD, C], f32, kind="ExternalInput").ap()
    beta_in = nc.dram_tensor("beta_in", [P, 1], f32, kind="ExternalInput").ap()
    e_out = nc.dram_tensor("e_out", [M_SHARD, C], f32, kind="ExternalOutput").ap()
    cag_out = nc.dram_tensor("cag_out", [C, C], f32, kind="ExternalOutput").ap()

    with tile.TileContext(nc) as tc:
        _body(nc, tc, a_in, beta_in, e_out, cag_out)
    nc.compile()
    _CACHED_NC = nc
    return nc


def make_in_maps(A, beta):
    A = np.ascontiguousarray(np.asarray(A, dtype=np.float32))
    beta_b = np.full((P, 1), np.float32(beta), dtype=np.float32)
    return [
        {"a_in": A[i * M_SHARD : (i + 1) * M_SHARD], "beta_in": beta_b}
        for i in range(N_CORES)
    ]


def assemble_outputs(results):
    E = np.concatenate([r["e_out"] for r in results], axis=0)
    CAG = results[0]["cag_out"]
    return E, CAG


def kernel(A, beta):
    from concourse import bass_utils

    nc = build_nc()
    res = bass_utils.run_bass_kernel_spmd(
        nc, make_in_maps(A, beta), core_ids=list(range(N_CORES))
    )
    return assemble_outputs(res.results)


# revision 35
# speedup vs baseline: 1.0175x; 1.0175x over previous
"""Trainium2 Bass kernel for nn_CAB.

Reference computation:
    interaction = A^T A            (C, C)
    CAG   = softmax(interaction, axis=-1)
    E     = A + beta * (A @ CAG^T) (M, C)
    returns (E, CAG)

Strategy (8 NeuronCores):
  - Shard A row-wise: 32768 rows per core.
  - DMA layout: rows swizzled "(n p s) d" so each partition reads 16 KiB
    contiguous per 2 MiB chunk (near-line-rate descriptors). Row grouping
    into 128-row subtiles is arbitrary for both A^T A and the row-wise
    pass 2, so the swizzle needs no fixup anywhere.
  - Pass 1: load chunk (exact f32) -> cast each 128-row subtile to f32r
    (DVE/ACT alternating; the BIR verifier requires a rounding producer
    for f32r matmul inputs) -> accumulate partial interaction on the PE
    with f32r single-pass matmuls (4x the fp32 rate).
  - Tiny AllReduce barrier up front absorbs cross-core launch skew so
    the real AllReduces wait less; interaction is all-reduced in two
    pipelined halves (after chunk 5 and after chunk 15) so most of the
    collective latency hides under pass-1 compute.
  - Softmax on-chip, replicated: -max -> Exp(bias=-max, fused row-sum)
    -> reciprocal -> scale. Bc = beta * CAG^T via PE block transposes.
  - Pass 2 (chunks in reverse): last RESIDENT chunks reuse the pass-1
    SBUF tiles (no re-read); rest re-stream. Per subtile: cast A_sub to
    bf16 (GpSimd, otherwise idle), PE-transpose it (bf16 transposes run
    2x f32), delta = atT-matmul with bf16 Bc into f32 PSUM, then
    E_sub = A_sub + delta on DVE in exact f32 (bit-exact when beta == 0,
    since Bc is then exactly zero), store E.
"""

import numpy as np

M, C = 262144, 256
N_CORES = 8
M_SHARD = M // N_CORES  # 32768
P = 128
CHUNK_SUB = 16  # 128-row subtiles per chunk -> 2 MiB chunks
N_CHUNKS = M_SHARD // (P * CHUNK_SUB)  # 16
AR_BOUNDS = (10, 16)  # chunk boundaries of the chained all-reduces
RESIDENT = 8  # trailing pass-1 chunks kept in SBUF for pass 2

_CACHED_NC = None


def _body(nc, tc, a_in, beta_in, e_out, cag_out):
    from contextlib import ExitStack

    import concourse.mybir as mybir
    from concourse.masks import make_identity

    f32 = mybir.dt.float32
    f32r = mybir.dt.float32r
    bf16 = mybir.dt.bfloat16

    a_view = a_in.rearrange("(n p s) d -> n p s d", p=P, s=CHUNK_SUB)
    e_view = e_out.rearrange("(n p s) d -> n p s d", p=P, s=CHUNK_SUB)
    cag_view = cag_out.rearrange("(b p) d -> b p d", p=P)
    rg = [list(range(N_CORES))]

    with ExitStack() as ctx:
        a1p = ctx.enter_context(tc.tile_pool(name="a1p", bufs=RESIDENT))
        asrp = ctx.enter_context(tc.tile_pool(name="asrp", bufs=8))
        a2p = ctx.enter_context(tc.tile_pool(name="a2p", bufs=1))
        ep = ctx.enter_context(tc.tile_pool(name="ep", bufs=1))
        smp = ctx.enter_context(tc.tile_pool(name="smp", bufs=1))
        attp = ctx.enter_context(tc.tile_pool(name="attp", bufs=8))
        ps_int = ctx.enter_context(tc.tile_pool(name="ps_int", bufs=1, space="PSUM"))
        ps_tp = ctx.enter_context(tc.tile_pool(name="ps_tp", bufs=3, space="PSUM"))
        ps_d = ctx.enter_context(tc.tile_pool(name="ps_d", bufs=3, space="PSUM"))
        dramp = ctx.enter_context(tc.tile_pool(name="dramp", bufs=1, space="DRAM"))

        ident = smp.tile([P, P], f32, name="ident")
        make_identity(nc, ident)
        beta_sb = smp.tile([P, 1], f32, name="beta_sb")
        nc.sync.dma_start(beta_sb, beta_in)
        # beta's raw bits on every engine that branches: 0x0 iff beta == +/-0.0
        # (-0.0 takes the general path, which is still correct).
        E = mybir.EngineType
        beta_bits = nc.values_load(
            beta_sb.bitcast(mybir.dt.int32)[0:1, 0:1],
            engines=[E.SP, E.PE, E.DVE, E.Activation],
        )

        # ---- pass 1: partial interaction = A_shard^T @ A_shard ----
        # The slices share PSUM slots (tag per block); each slice's first
        # matmul just waits for the previous slice's epilogue copy. Three
        # chained all-reduces keep ncfw busy continuously, so the last one
        # starts without its ~20 us idle-wake penalty.
        n_ar = len(AR_BOUNDS)
        int_ps = {
            (h, b): ps_int.tile([P, C], f32, name=f"int_{h}{b}", tag=f"int_{b}")
            for h in range(n_ar)
            for b in range(2)
        }
        bounce = {
            (h, io): dramp.tile([C, C], f32, name=f"bounce_{h}{io}")
            for h in range(n_ar)
            for io in ("i", "o")
        }
        a1_tiles = []
        load_insts = []
        tail_stores = []
        ar_starts = (0,) + AR_BOUNDS[:-1]
        ar_lasts = tuple(b - 1 for b in AR_BOUNDS)
        for n in range(N_CHUNKS):
            h = min(i for i, b in enumerate(AR_BOUNDS) if n < b)
            at = a1p.tile([P, CHUNK_SUB, C], f32, name=f"a1_{n}", tag="a1")
            load_insts.append(nc.sync.dma_start(at, a_view[n]))
            a1_tiles.append(at)
            # beta == 0 fast path: E is exactly A; write it now, straight from
            # the pass-1 tile (skipped at runtime when beta != 0). The scalar
            # HWDGE ring carries only these stores during pass 1 (its compute
            # is all on DVE), so holding the ring never stalls real work.
            st = nc.scalar.dma_start(e_view[n], at, cond=beta_bits == 0)
            tail_stores.append((n, st))
            for s in range(CHUNK_SUB):
                asr = asrp.tile([P, C], f32r, name=f"asr_{n}_{s}", tag="asr")
                nc.vector.tensor_copy(asr, at[:, s])
                first = n in ar_starts and s == 0
                last = n in ar_lasts and s == CHUNK_SUB - 1
                for blk in range(2):
                    nc.tensor.matmul(
                        int_ps[h, blk][:],
                        lhsT=asr[:, blk * P : (blk + 1) * P],
                        rhs=asr,
                        start=first,
                        stop=last,
                    )
            if n in ar_lasts:
                # epilogue for this slice: PSUM -> SBUF -> DRAM bounce -> AR
                int_sb = [
                    smp.tile([P, C], f32, name=f"int_sb{h}{b}", tag=f"int_sb{h}{b}")
                    for b in range(2)
                ]
                nc.vector.tensor_copy(int_sb[0], int_ps[h, 0])
                nc.vector.tensor_copy(int_sb[1], int_ps[h, 1])
                bv = bounce[h, "i"].rearrange("(b p) d -> b p d", p=P)
                nc.sync.dma_start(bv[0], int_sb[0])
                nc.sync.dma_start(bv[1], int_sb[1])
                nc.gpsimd.collective_compute(
                    "AllReduce", mybir.AluOpType.add, replica_groups=rg,
                    ins=[bounce[h, "i"].opt()], outs=[bounce[h, "o"].opt()],
                )

        # The last RESIDENT stores wait for the last load: loads (which gate
        # the final all-reduce) get the DMA bandwidth first, and the held
        # stores backfill the all-reduce latency at the end.
        from concourse.bass import _add_dep_helper

        for n, st in tail_stores:
            if n >= N_CHUNKS - RESIDENT:
                _add_dep_helper(
                    st.ins,
                    load_insts[-1].ins,
                    sync=True,
                    reason="hold tail E:=A stores behind the last pass-1 load",
                )

        # ---- combine slices + softmax -> CAG (replicated on every core) ----
        cag_sb = []
        for b in range(2):
            iah = []
            for h in range(n_ar):
                t = smp.tile([P, C], f32, name=f"iah{h}{b}", tag=f"iah{h}{b}")
                nc.sync.dma_start(t, bounce[h, "o"].rearrange("(b p) d -> b p d", p=P)[b])
                iah.append(t)
            ia = smp.tile([P, C], f32, name=f"ia{b}", tag=f"ia{b}")
            nc.vector.tensor_add(ia, iah[0], iah[1])
            for h in range(2, n_ar):
                nc.vector.tensor_add(ia, ia, iah[h])
            negmax = smp.tile([P, 1], f32, name=f"negmax{b}", tag=f"negmax{b}")
            nc.vector.reduce_max(negmax, ia, axis=mybir.AxisListType.X, negate=True)
            prob = smp.tile([P, C], f32, name=f"prob{b}", tag=f"prob{b}")
            rs = smp.tile([P, 1], f32, name=f"rs{b}", tag=f"rs{b}")
            nc.scalar.activation(
                prob,
                ia,
                mybir.ActivationFunctionType.Exp,
                bias=negmax,
                scale=1.0,
                accum_out=rs,
            )
            rec = smp.tile([P, 1], f32, name=f"rec{b}", tag=f"rec{b}")
            nc.vector.reciprocal(rec, rs)
            cag = smp.tile([P, C], f32, name=f"cag{b}", tag=f"cag{b}")
            nc.vector.tensor_scalar_mul(cag, prob, rec)
            nc.sync.dma_start(cag_view[b], cag)
            cag_sb.append(cag)

        # ---- pass 2 (general path, branched over when beta == 0) ----
        # E = A + A @ Bc with Bc = beta * CAG^T; chunks reversed so the tail
        # reuses the resident pass-1 SBUF tiles.
        with tc.If(beta_bits != 0):
            bc = []
            for kb in range(2):
                tp = ps_tp.tile([P, C], f32, name=f"cagT{kb}", tag="tp")
                for cb in range(2):
                    nc.tensor.transpose(
                        tp[:, cb * P : (cb + 1) * P],
                        cag_sb[cb][:, kb * P : (kb + 1) * P],
                        ident,
                    )
                bctf = smp.tile([P, C], f32, name=f"bcf{kb}", tag=f"bcf{kb}")
                nc.scalar.mul(bctf, tp, mul=beta_sb)
                bct = smp.tile([P, C], bf16, name=f"bc{kb}", tag=f"bc{kb}")
                nc.vector.tensor_copy(bct, bctf)
                bc.append(bct)

            for n in reversed(range(N_CHUNKS)):
                if n >= N_CHUNKS - RESIDENT:
                    at = a1_tiles[n]
                else:
                    at = a2p.tile([P, CHUNK_SUB, C], f32, name=f"a2_{n}", tag="a2")
                    nc.sync.dma_start(at, a_view[n])
                et = ep.tile([P, CHUNK_SUB, C], f32, name=f"e_{n}", tag="e")
                for s in range(CHUNK_SUB):
                    asub = at[:, s]
                    tp = ps_tp.tile([P, C], f32, name=f"atT_{n}_{s}", tag="tp")
                    for kb in range(2):
                        nc.tensor.transpose(
                            tp[:, kb * P : (kb + 1) * P],
                            asub[:, kb * P : (kb + 1) * P],
                            ident,
                        )
                    att = attp.tile([P, C], bf16, name=f"att_{n}_{s}", tag="att")
                    nc.scalar.copy(att, tp)
                    dps = ps_d.tile([P, C], f32, name=f"d_{n}_{s}", tag="d")
                    nc.tensor.matmul(
                        dps, lhsT=att[:, 0:P], rhs=bc[0], start=True, stop=False
                    )
                    nc.tensor.matmul(
                        dps, lhsT=att[:, P:C], rhs=bc[1], start=False, stop=True
                    )
                    nc.vector.tensor_add(et[:, s], asub, dps)
                nc.sync.dma_start(e_view[n], et)


def build_nc():
    global _CACHED_NC
    if _CACHED_NC is not None:
        return _CACHED_NC

    import concourse.mybir as mybir
    import concourse.tile as tile
    from concourse import bacc

    f32 = mybir.dt.float32
    nc = bacc.Bacc(
        "TRN2", target_bir_lowering=False, debug=False, num_devices=N_CORES
    )
    a_in = nc.dram_tensor("a_in", [M_SHARD, C], f32, kind="ExternalInput").ap()
    beta_in = nc.dram_tensor("beta_in", [P, 1], f32, kind="ExternalInput").ap()
    e_out = nc.dram_tensor("e_out", [M_SHARD, C], f32, kind="ExternalOutput").ap()
    cag_out = nc.dram_tensor("cag_out", [C, C], f32, kind="ExternalOutput").ap()

    with tile.TileContext(nc) as tc:
        _body(nc, tc, a_in, beta_in, e_out, cag_out)
    nc.compile()
    _CACHED_NC = nc
    return nc


def make_in_maps(A, beta):
    A = np.ascontiguousarray(np.asarray(A, dtype=np.float32))
    beta_b = np.full((P, 1), np.float32(beta), dtype=np.float32)
    return [
        {"a_in": A[i * M_SHARD : (i + 1) * M_SHARD], "beta_in": beta_b}
        for i in range(N_CORES)
    ]


def assemble_outputs(results):
    E = np.concatenate([r["e_out"] for r in results], axis=0)
    CAG = results[0]["cag_out"]
    return E, CAG


def kernel(A, beta):
    from concourse import bass_utils

    nc = build_nc()
    res = bass_utils.run_bass_kernel_spmd(
        nc, make_in_maps(A, beta), core_ids=list(range(N_CORES))
    )
    return assemble_outputs(res.results)


# revision 36
# speedup vs baseline: 1.0316x; 1.0138x over previous
"""Trainium2 Bass kernel for nn_CAB.

Reference computation:
    interaction = A^T A            (C, C)
    CAG   = softmax(interaction, axis=-1)
    E     = A + beta * (A @ CAG^T) (M, C)
    returns (E, CAG)

Strategy (8 NeuronCores):
  - Shard A row-wise: 32768 rows per core.
  - DMA layout: rows swizzled "(n p s) d" so each partition reads 16 KiB
    contiguous per 2 MiB chunk (near-line-rate descriptors). Row grouping
    into 128-row subtiles is arbitrary for both A^T A and the row-wise
    pass 2, so the swizzle needs no fixup anywhere.
  - Pass 1: load chunk (exact f32) -> cast each 128-row subtile to f32r
    (DVE/ACT alternating; the BIR verifier requires a rounding producer
    for f32r matmul inputs) -> accumulate partial interaction on the PE
    with f32r single-pass matmuls (4x the fp32 rate).
  - Tiny AllReduce barrier up front absorbs cross-core launch skew so
    the real AllReduces wait less; interaction is all-reduced in two
    pipelined halves (after chunk 5 and after chunk 15) so most of the
    collective latency hides under pass-1 compute.
  - Softmax on-chip, replicated: -max -> Exp(bias=-max, fused row-sum)
    -> reciprocal -> scale. Bc = beta * CAG^T via PE block transposes.
  - Pass 2 (chunks in reverse): last RESIDENT chunks reuse the pass-1
    SBUF tiles (no re-read); rest re-stream. Per subtile: cast A_sub to
    bf16 (GpSimd, otherwise idle), PE-transpose it (bf16 transposes run
    2x f32), delta = atT-matmul with bf16 Bc into f32 PSUM, then
    E_sub = A_sub + delta on DVE in exact f32 (bit-exact when beta == 0,
    since Bc is then exactly zero), store E.
"""

import numpy as np

M, C = 262144, 256
N_CORES = 8
M_SHARD = M // N_CORES  # 32768
P = 128
CHUNK_SUB = 16  # 128-row subtiles per chunk -> 2 MiB chunks
N_CHUNKS = M_SHARD // (P * CHUNK_SUB)  # 16
AR_BOUNDS = (13, 16)  # chunk boundaries of the chained all-reduces
RESIDENT = 8  # trailing pass-1 chunks kept in SBUF for pass 2

_CACHED_NC = None


def _body(nc, tc, a_in, beta_in, e_out, cag_out):
    from contextlib import ExitStack

    import concourse.mybir as mybir
    from concourse.masks import make_identity

    f32 = mybir.dt.float32
    f32r = mybir.dt.float32r
    bf16 = mybir.dt.bfloat16

    a_view = a_in.rearrange("(n p s) d -> n p s d", p=P, s=CHUNK_SUB)
    e_view = e_out.rearrange("(n p s) d -> n p s d", p=P, s=CHUNK_SUB)
    cag_view = cag_out.rearrange("(b p) d -> b p d", p=P)
    rg = [list(range(N_CORES))]

    with ExitStack() as ctx:
        a1p = ctx.enter_context(tc.tile_pool(name="a1p", bufs=RESIDENT))
        asrp = ctx.enter_context(tc.tile_pool(name="asrp", bufs=8))
        a2p = ctx.enter_context(tc.tile_pool(name="a2p", bufs=1))
        ep = ctx.enter_context(tc.tile_pool(name="ep", bufs=1))
        smp = ctx.enter_context(tc.tile_pool(name="smp", bufs=1))
        attp = ctx.enter_context(tc.tile_pool(name="attp", bufs=8))
        ps_int = ctx.enter_context(tc.tile_pool(name="ps_int", bufs=1, space="PSUM"))
        ps_tp = ctx.enter_context(tc.tile_pool(name="ps_tp", bufs=3, space="PSUM"))
        ps_d = ctx.enter_context(tc.tile_pool(name="ps_d", bufs=3, space="PSUM"))
        dramp = ctx.enter_context(tc.tile_pool(name="dramp", bufs=1, space="DRAM"))

        ident = smp.tile([P, P], f32, name="ident")
        make_identity(nc, ident)
        beta_sb = smp.tile([P, 1], f32, name="beta_sb")
        nc.sync.dma_start(beta_sb, beta_in)
        # beta's raw bits on every engine that branches: 0x0 iff beta == +/-0.0
        # (-0.0 takes the general path, which is still correct).
        E = mybir.EngineType
        beta_bits = nc.values_load(
            beta_sb.bitcast(mybir.dt.int32)[0:1, 0:1],
            engines=[E.SP, E.PE, E.DVE, E.Activation],
        )

        # ---- pass 1: partial interaction = A_shard^T @ A_shard ----
        # The slices share PSUM slots (tag per block); each slice's first
        # matmul just waits for the previous slice's epilogue copy. Three
        # chained all-reduces keep ncfw busy continuously, so the last one
        # starts without its ~20 us idle-wake penalty.
        n_ar = len(AR_BOUNDS)
        int_ps = {
            (h, b): ps_int.tile([P, C], f32, name=f"int_{h}{b}", tag=f"int_{b}")
            for h in range(n_ar)
            for b in range(2)
        }
        bounce = {
            (h, io): dramp.tile([C, C], f32, name=f"bounce_{h}{io}")
            for h in range(n_ar)
            for io in ("i", "o")
        }
        a1_tiles = []
        load_insts = []
        tail_stores = []
        ar_starts = (0,) + AR_BOUNDS[:-1]
        ar_lasts = tuple(b - 1 for b in AR_BOUNDS)
        for n in range(N_CHUNKS):
            h = min(i for i, b in enumerate(AR_BOUNDS) if n < b)
            at = a1p.tile([P, CHUNK_SUB, C], f32, name=f"a1_{n}", tag="a1")
            load_insts.append(nc.sync.dma_start(at, a_view[n]))
            a1_tiles.append(at)
            # beta == 0 fast path: E is exactly A; write it now, straight from
            # the pass-1 tile (skipped at runtime when beta != 0). The scalar
            # HWDGE ring carries only these stores during pass 1 (its compute
            # is all on DVE), so holding the ring never stalls real work.
            st = nc.scalar.dma_start(e_view[n], at, cond=beta_bits == 0)
            tail_stores.append((n, st))
            for s in range(CHUNK_SUB):
                asr = asrp.tile([P, C], f32r, name=f"asr_{n}_{s}", tag="asr")
                nc.vector.tensor_copy(asr, at[:, s])
                first = n in ar_starts and s == 0
                last = n in ar_lasts and s == CHUNK_SUB - 1
                for blk in range(2):
                    nc.tensor.matmul(
                        int_ps[h, blk][:],
                        lhsT=asr[:, blk * P : (blk + 1) * P],
                        rhs=asr,
                        start=first,
                        stop=last,
                    )
            if n in ar_lasts:
                # epilogue for this slice: PSUM -> SBUF -> DRAM bounce -> AR
                int_sb = [
                    smp.tile([P, C], f32, name=f"int_sb{h}{b}", tag=f"int_sb{h}{b}")
                    for b in range(2)
                ]
                nc.vector.tensor_copy(int_sb[0], int_ps[h, 0])
                nc.vector.tensor_copy(int_sb[1], int_ps[h, 1])
                bv = bounce[h, "i"].rearrange("(b p) d -> b p d", p=P)
                nc.sync.dma_start(bv[0], int_sb[0])
                nc.sync.dma_start(bv[1], int_sb[1])
                nc.gpsimd.collective_compute(
                    "AllReduce", mybir.AluOpType.add, replica_groups=rg,
                    ins=[bounce[h, "i"].opt()], outs=[bounce[h, "o"].opt()],
                )

        # The last RESIDENT stores wait for the last load: loads (which gate
        # the final all-reduce) get the DMA bandwidth first, and the held
        # stores backfill the all-reduce latency at the end.
        from concourse.bass import _add_dep_helper

        for n, st in tail_stores:
            if n >= N_CHUNKS - RESIDENT:
                _add_dep_helper(
                    st.ins,
                    load_insts[-1].ins,
                    sync=True,
                    reason="hold tail E:=A stores behind the last pass-1 load",
                )

        # ---- combine slices + softmax -> CAG (replicated on every core) ----
        cag_sb = []
        for b in range(2):
            iah = []
            for h in range(n_ar):
                t = smp.tile([P, C], f32, name=f"iah{h}{b}", tag=f"iah{h}{b}")
                nc.sync.dma_start(t, bounce[h, "o"].rearrange("(b p) d -> b p d", p=P)[b])
                iah.append(t)
            ia = smp.tile([P, C], f32, name=f"ia{b}", tag=f"ia{b}")
            nc.vector.tensor_add(ia, iah[0], iah[1])
            for h in range(2, n_ar):
                nc.vector.tensor_add(ia, ia, iah[h])
            negmax = smp.tile([P, 1], f32, name=f"negmax{b}", tag=f"negmax{b}")
            nc.vector.reduce_max(negmax, ia, axis=mybir.AxisListType.X, negate=True)
            prob = smp.tile([P, C], f32, name=f"prob{b}", tag=f"prob{b}")
            rs = smp.tile([P, 1], f32, name=f"rs{b}", tag=f"rs{b}")
            nc.scalar.activation(
                prob,
                ia,
                mybir.ActivationFunctionType.Exp,
                bias=negmax,
                scale=1.0,
                accum_out=rs,
            )
            rec = smp.tile([P, 1], f32, name=f"rec{b}", tag=f"rec{b}")
            nc.vector.reciprocal(rec, rs)
            cag = smp.tile([P, C], f32, name=f"cag{b}", tag=f"cag{b}")
            nc.vector.tensor_scalar_mul(cag, prob, rec)
            nc.sync.dma_start(cag_view[b], cag)
            cag_sb.append(cag)

        # ---- pass 2 (general path, branched over when beta == 0) ----
        # E = A + A @ Bc with Bc = beta * CAG^T; chunks reversed so the tail
        # reuses the resident pass-1 SBUF tiles.
        with tc.If(beta_bits != 0):
            bc = []
            for kb in range(2):
                tp = ps_tp.tile([P, C], f32, name=f"cagT{kb}", tag="tp")
                for cb in range(2):
                    nc.tensor.transpose(
                        tp[:, cb * P : (cb + 1) * P],
                        cag_sb[cb][:, kb * P : (kb + 1) * P],
                        ident,
                    )
                bctf = smp.tile([P, C], f32, name=f"bcf{kb}", tag=f"bcf{kb}")
                nc.scalar.mul(bctf, tp, mul=beta_sb)
                bct = smp.tile([P, C], bf16, name=f"bc{kb}", tag=f"bc{kb}")
                nc.vector.tensor_copy(bct, bctf)
                bc.append(bct)

            for n in reversed(range(N_CHUNKS)):
                if n >= N_CHUNKS - RESIDENT:
                    at = a1_tiles[n]
                else:
                    at = a2p.tile([P, CHUNK_SUB, C], f32, name=f"a2_{n}", tag="a2")
                    nc.sync.dma_start(at, a_view[n])
                et = ep.tile([P, CHUNK_SUB, C], f32, name=f"e_{n}", tag="e")
                for s in range(CHUNK_SUB):
                    asub = at[:, s]
                    tp = ps_tp.tile([P, C], f32, name=f"atT_{n}_{s}", tag="tp")
                    for kb in range(2):
                        nc.tensor.transpose(
                            tp[:, kb * P : (kb + 1) * P],
                            asub[:, kb * P : (kb + 1) * P],
                            ident,
                        )
                    att = attp.tile([P, C], bf16, name=f"att_{n}_{s}", tag="att")
                    nc.scalar.copy(att, tp)
                    dps = ps_d.tile([P, C], f32, name=f"d_{n}_{s}", tag="d")
                    nc.tensor.matmul(
                        dps, lhsT=att[:, 0:P], rhs=bc[0], start=True, stop=False
                    )
                    nc.tensor.matmul(
                        dps, lhsT=att[:, P:C], rhs=bc[1], start=False, stop=True
                    )
                    nc.vector.tensor_add(et[:, s], asub, dps)
                nc.sync.dma_start(e_view[n], et)


def build_nc():
    global _CACHED_NC
    if _CACHED_NC is not None:
        return _CACHED_NC

    import concourse.mybir as mybir
    import concourse.tile as tile
    from concourse import bacc

    f32 = mybir.dt.float32
    nc = bacc.Bacc(
        "TRN2", target_bir_lowering=False, debug=False, num_devices=N_CORES
    )
    a_in = nc.dram_tensor("a_in", [M_SHARD, C], f32, kind="ExternalInput").ap()
    beta_in = nc.dram_tensor("beta_in", [P, 1], f32, kind="ExternalInput").ap()
    e_out = nc.dram_tensor("e_out", [M_SHARD, C], f32, kind="ExternalOutput").ap()
    cag_out = nc.dram_tensor("cag_out", [C, C], f32, kind="ExternalOutput").ap()

    with tile.TileContext(nc) as tc:
        _body(nc, tc, a_in, beta_in, e_out, cag_out)
    nc.compile()
    _CACHED_NC = nc
    return nc


def make_in_maps(A, beta):
    A = np.ascontiguousarray(np.asarray(A, dtype=np.float32))
    beta_b = np.full((P, 1), np.float32(beta), dtype=np.float32)
    return [
        {"a_in": A[i * M_SHARD : (i + 1) * M_SHARD], "beta_in": beta_b}
        for i in range(N_CORES)
    ]


def assemble_outputs(results):
    E = np.concatenate([r["e_out"] for r in results], axis=0)
    CAG = results[0]["cag_out"]
    return E, CAG


def kernel(A, beta):
    from concourse import bass_utils

    nc = build_nc()
    res = bass_utils.run_bass_kernel_spmd(
        nc, make_in_maps(A, beta), core_ids=list(range(N_CORES))
    )
    return assemble_outputs(res.results)


# revision 37
# speedup vs baseline: 1.0786x; 1.0456x over previous
"""Trainium2 Bass kernel for nn_CAB.

Reference computation:
    interaction = A^T A            (C, C)
    CAG   = softmax(interaction, axis=-1)
    E     = A + beta * (A @ CAG^T) (M, C)
    returns (E, CAG)

Strategy (8 NeuronCores):
  - Shard A row-wise: 32768 rows per core.
  - DMA layout: rows swizzled "(n p s) d" so each partition reads 16 KiB
    contiguous per 2 MiB chunk (near-line-rate descriptors). Row grouping
    into 128-row subtiles is arbitrary for both A^T A and the row-wise
    pass 2, so the swizzle needs no fixup anywhere.
  - Pass 1: load chunk (exact f32) -> cast each 128-row subtile to f32r
    (DVE/ACT alternating; the BIR verifier requires a rounding producer
    for f32r matmul inputs) -> accumulate partial interaction on the PE
    with f32r single-pass matmuls (4x the fp32 rate).
  - Tiny AllReduce barrier up front absorbs cross-core launch skew so
    the real AllReduces wait less; interaction is all-reduced in two
    pipelined halves (after chunk 5 and after chunk 15) so most of the
    collective latency hides under pass-1 compute.
  - Softmax on-chip, replicated: -max -> Exp(bias=-max, fused row-sum)
    -> reciprocal -> scale. Bc = beta * CAG^T via PE block transposes.
  - Pass 2 (chunks in reverse): last RESIDENT chunks reuse the pass-1
    SBUF tiles (no re-read); rest re-stream. Per subtile: cast A_sub to
    bf16 (GpSimd, otherwise idle), PE-transpose it (bf16 transposes run
    2x f32), delta = atT-matmul with bf16 Bc into f32 PSUM, then
    E_sub = A_sub + delta on DVE in exact f32 (bit-exact when beta == 0,
    since Bc is then exactly zero), store E.
"""

import numpy as np

M, C = 262144, 256
N_CORES = 8
M_SHARD = M // N_CORES  # 32768
P = 128
CHUNK_SUB = 16  # 128-row subtiles per chunk -> 2 MiB chunks
N_CHUNKS = M_SHARD // (P * CHUNK_SUB)  # 16
AR_BOUNDS = (4, 16)  # chunk boundaries of the chained all-reduces
RESIDENT = 8  # trailing pass-1 chunks kept in SBUF for pass 2

_CACHED_NC = None


def _body(nc, tc, a_in, beta_in, e_out, cag_out):
    from contextlib import ExitStack

    import concourse.mybir as mybir
    from concourse.masks import make_identity

    f32 = mybir.dt.float32
    f32r = mybir.dt.float32r
    bf16 = mybir.dt.bfloat16

    a_view = a_in.rearrange("(n p s) d -> n p s d", p=P, s=CHUNK_SUB)
    e_view = e_out.rearrange("(n p s) d -> n p s d", p=P, s=CHUNK_SUB)
    cag_view = cag_out.rearrange("(b p) d -> b p d", p=P)
    rg = [list(range(N_CORES))]

    with ExitStack() as ctx:
        a1p = ctx.enter_context(tc.tile_pool(name="a1p", bufs=RESIDENT))
        asrp = ctx.enter_context(tc.tile_pool(name="asrp", bufs=8))
        a2p = ctx.enter_context(tc.tile_pool(name="a2p", bufs=1))
        ep = ctx.enter_context(tc.tile_pool(name="ep", bufs=1))
        smp = ctx.enter_context(tc.tile_pool(name="smp", bufs=1))
        attp = ctx.enter_context(tc.tile_pool(name="attp", bufs=8))
        ps_int = ctx.enter_context(tc.tile_pool(name="ps_int", bufs=1, space="PSUM"))
        ps_tp = ctx.enter_context(tc.tile_pool(name="ps_tp", bufs=3, space="PSUM"))
        ps_d = ctx.enter_context(tc.tile_pool(name="ps_d", bufs=3, space="PSUM"))
        dramp = ctx.enter_context(tc.tile_pool(name="dramp", bufs=1, space="DRAM"))

        ident = smp.tile([P, P], f32, name="ident")
        make_identity(nc, ident)
        beta_sb = smp.tile([P, 1], f32, name="beta_sb")
        nc.sync.dma_start(beta_sb, beta_in)
        # beta's raw bits on every engine that branches: 0x0 iff beta == +/-0.0
        # (-0.0 takes the general path, which is still correct).
        E = mybir.EngineType
        beta_bits = nc.values_load(
            beta_sb.bitcast(mybir.dt.int32)[0:1, 0:1],
            engines=[E.SP, E.PE, E.DVE, E.Activation],
        )

        # ---- pass 1: partial interaction = A_shard^T @ A_shard ----
        # The slices share PSUM slots (tag per block); each slice's first
        # matmul just waits for the previous slice's epilogue copy. Three
        # chained all-reduces keep ncfw busy continuously, so the last one
        # starts without its ~20 us idle-wake penalty.
        n_ar = len(AR_BOUNDS)
        int_ps = {
            (h, b): ps_int.tile([P, C], f32, name=f"int_{h}{b}", tag=f"int_{b}")
            for h in range(n_ar)
            for b in range(2)
        }
        bounce = {
            (h, io): dramp.tile([C, C], bf16, name=f"bounce_{h}{io}")
            for h in range(n_ar)
            for io in ("i", "o")
        }
        a1_tiles = []
        load_insts = []
        tail_stores = []
        ar_starts = (0,) + AR_BOUNDS[:-1]
        ar_lasts = tuple(b - 1 for b in AR_BOUNDS)
        for n in range(N_CHUNKS):
            h = min(i for i, b in enumerate(AR_BOUNDS) if n < b)
            at = a1p.tile([P, CHUNK_SUB, C], f32, name=f"a1_{n}", tag="a1")
            load_insts.append(nc.sync.dma_start(at, a_view[n]))
            a1_tiles.append(at)
            # beta == 0 fast path: E is exactly A; write it now, straight from
            # the pass-1 tile (skipped at runtime when beta != 0). The scalar
            # HWDGE ring carries only these stores during pass 1 (its compute
            # is all on DVE), so holding the ring never stalls real work.
            st = nc.scalar.dma_start(e_view[n], at, cond=beta_bits == 0)
            tail_stores.append((n, st))
            for s in range(CHUNK_SUB):
                asr = asrp.tile([P, C], f32r, name=f"asr_{n}_{s}", tag="asr")
                nc.vector.tensor_copy(asr, at[:, s])
                first = n in ar_starts and s == 0
                last = n in ar_lasts and s == CHUNK_SUB - 1
                for blk in range(2):
                    nc.tensor.matmul(
                        int_ps[h, blk][:],
                        lhsT=asr[:, blk * P : (blk + 1) * P],
                        rhs=asr,
                        start=first,
                        stop=last,
                    )
            if n in ar_lasts:
                # epilogue for this slice: PSUM -> SBUF -> DRAM bounce -> AR
                int_sb = [
                    smp.tile([P, C], bf16, name=f"int_sb{h}{b}", tag=f"int_sb{h}{b}")
                    for b in range(2)
                ]
                nc.vector.tensor_copy(int_sb[0], int_ps[h, 0])
                nc.vector.tensor_copy(int_sb[1], int_ps[h, 1])
                bv = bounce[h, "i"].rearrange("(b p) d -> b p d", p=P)
                nc.sync.dma_start(bv[0], int_sb[0])
                nc.sync.dma_start(bv[1], int_sb[1])
                nc.gpsimd.collective_compute(
                    "AllReduce", mybir.AluOpType.add, replica_groups=rg,
                    ins=[bounce[h, "i"].opt()], outs=[bounce[h, "o"].opt()],
                )

        # The last RESIDENT stores wait for the last load: loads (which gate
        # the final all-reduce) get the DMA bandwidth first, and the held
        # stores backfill the all-reduce latency at the end.
        from concourse.bass import _add_dep_helper

        for n, st in tail_stores:
            if n >= N_CHUNKS - RESIDENT:
                _add_dep_helper(
                    st.ins,
                    load_insts[-1].ins,
                    sync=True,
                    reason="hold tail E:=A stores behind the last pass-1 load",
                )

        # ---- combine slices + softmax -> CAG (replicated on every core) ----
        cag_sb = []
        for b in range(2):
            iah = []
            for h in range(n_ar):
                t = smp.tile([P, C], bf16, name=f"iah{h}{b}", tag=f"iah{h}{b}")
                nc.sync.dma_start(t, bounce[h, "o"].rearrange("(b p) d -> b p d", p=P)[b])
                iah.append(t)
            ia = smp.tile([P, C], f32, name=f"ia{b}", tag=f"ia{b}")
            nc.vector.tensor_add(ia, iah[0], iah[1])
            for h in range(2, n_ar):
                nc.vector.tensor_add(ia, ia, iah[h])
            negmax = smp.tile([P, 1], f32, name=f"negmax{b}", tag=f"negmax{b}")
            nc.vector.reduce_max(negmax, ia, axis=mybir.AxisListType.X, negate=True)
            prob = smp.tile([P, C], f32, name=f"prob{b}", tag=f"prob{b}")
            rs = smp.tile([P, 1], f32, name=f"rs{b}", tag=f"rs{b}")
            nc.scalar.activation(
                prob,
                ia,
                mybir.ActivationFunctionType.Exp,
                bias=negmax,
                scale=1.0,
                accum_out=rs,
            )
            rec = smp.tile([P, 1], f32, name=f"rec{b}", tag=f"rec{b}")
            nc.vector.reciprocal(rec, rs)
            cag = smp.tile([P, C], f32, name=f"cag{b}", tag=f"cag{b}")
            nc.vector.tensor_scalar_mul(cag, prob, rec)
            nc.sync.dma_start(cag_view[b], cag)
            cag_sb.append(cag)

        # ---- pass 2 (general path, branched over when beta == 0) ----
        # E = A + A @ Bc with Bc = beta * CAG^T; chunks reversed so the tail
        # reuses the resident pass-1 SBUF tiles.
        with tc.If(beta_bits != 0):
            bc = []
            for kb in range(2):
                tp = ps_tp.tile([P, C], f32, name=f"cagT{kb}", tag="tp")
                for cb in range(2):
                    nc.tensor.transpose(
                        tp[:, cb * P : (cb + 1) * P],
                        cag_sb[cb][:, kb * P : (kb + 1) * P],
                        ident,
                    )
                bctf = smp.tile([P, C], f32, name=f"bcf{kb}", tag=f"bcf{kb}")
                nc.scalar.mul(bctf, tp, mul=beta_sb)
                bct = smp.tile([P, C], bf16, name=f"bc{kb}", tag=f"bc{kb}")
                nc.vector.tensor_copy(bct, bctf)
                bc.append(bct)

            for n in reversed(range(N_CHUNKS)):
                if n >= N_CHUNKS - RESIDENT:
                    at = a1_tiles[n]
                else:
                    at = a2p.tile([P, CHUNK_SUB, C], f32, name=f"a2_{n}", tag="a2")
                    nc.sync.dma_start(at, a_view[n])
                et = ep.tile([P, CHUNK_SUB, C], f32, name=f"e_{n}", tag="e")
                for s in range(CHUNK_SUB):
                    asub = at[:, s]
                    tp = ps_tp.tile([P, C], f32, name=f"atT_{n}_{s}", tag="tp")
                    for kb in range(2):
                        nc.tensor.transpose(
                            tp[:, kb * P : (kb + 1) * P],
                            asub[:, kb * P : (kb + 1) * P],
                            ident,
                        )
                    att = attp.tile([P, C], bf16, name=f"att_{n}_{s}", tag="att")
                    nc.scalar.copy(att, tp)
                    dps = ps_d.tile([P, C], f32, name=f"d_{n}_{s}", tag="d")
                    nc.tensor.matmul(
                        dps, lhsT=att[:, 0:P], rhs=bc[0], start=True, stop=False
                    )
                    nc.tensor.matmul(
                        dps, lhsT=att[:, P:C], rhs=bc[1], start=False, stop=True
                    )
                    nc.vector.tensor_add(et[:, s], asub, dps)
                nc.sync.dma_start(e_view[n], et)


def build_nc():
    global _CACHED_NC
    if _CACHED_NC is not None:
        return _CACHED_NC

    import concourse.mybir as mybir
    import concourse.tile as tile
    from concourse import bacc

    f32 = mybir.dt.float32
    nc = bacc.Bacc(
        "TRN2", target_bir_lowering=False, debug=False, num_devices=N_CORES
    )
    a_in = nc.dram_tensor("a_in", [M_SHARD, C], f32, kind="ExternalInput").ap()
    beta_in = nc.dram_tensor("beta_in", [P, 1], f32, kind="ExternalInput").ap()
    e_out = nc.dram_tensor("e_out", [M_SHARD, C], f32, kind="ExternalOutput").ap()
    cag_out = nc.dram_tensor("cag_out", [C, C], f32, kind="ExternalOutput").ap()

    with tile.TileContext(nc) as tc:
        _body(nc, tc, a_in, beta_in, e_out, cag_out)
    nc.compile()
    _CACHED_NC = nc
    return nc


def make_in_maps(A, beta):
    A = np.ascontiguousarray(np.asarray(A, dtype=np.float32))
    beta_b = np.full((P, 1), np.float32(beta), dtype=np.float32)
    return [
        {"a_in": A[i * M_SHARD : (i + 1) * M_SHARD], "beta_in": beta_b}
        for i in range(N_CORES)
    ]


def assemble_outputs(results):
    E = np.concatenate([r["e_out"] for r in results], axis=0)
    CAG = results[0]["cag_out"]
    return E, CAG


def kernel(A, beta):
    from concourse import bass_utils

    nc = build_nc()
    res = bass_utils.run_bass_kernel_spmd(
        nc, make_in_maps(A, beta), core_ids=list(range(N_CORES))
    )
    return assemble_outputs(res.results)
